# revision 22
# baseline (speedup 1.0000x reference)
"""Trainium2 Bass kernel for MockFP8Linear: out = x @ (W * block_scale)^T.

Strategy: data-parallel over tokens across 8 NeuronCores (no collectives).

Layout: the PE contracts along the partition dim, so both operands need
in_features on partitions. Both are fed to the device pre-transposed as
host-side layout prep (np.ascontiguousarray(.T) + bf16 cast, exactly the
prep class the baseline already used for W):
  - weight: [in, out] bf16. Dequant (per-128x128-block scale) happens
    on-device in one DVE tensor_tensor multiply per k-row, using a
    stride-0 broadcast AP for the scales. W^T (bf16, 8 MB) stays
    resident in SBUF.
  - x: tile-blocked transposed bf16 per-core shard, xb[t, p, kb, m] =
    x[t*128+m, kb*128+p], so each token tile is ONE [128, 4KB-run] DMA
    (DMA engines are packet-rate bound: 256B-run block DMAs measured
    ~6 GB/s/engine vs ~24 GB/s at 4KB runs) and lhsT blocks slice
    straight out of SBUF. No on-device transpose or cast: the
    TensorEngine runs a pure matmul stream.

Main compute runs as TWO PASSES over output halves so the prologue only
gates on half the W dequant: pass A computes out[:, 0:1024] for all 16
token tiles (the first four tiles interleaved k-block-by-k-block so the
PE chases the W-half-row DMA arrivals), pass B computes out[:, 1024:]
as a pure matmul stream over the fully resident operands. Per (tile,
k-block): lhsT(=x^T block, stationary) @ rhs(=W^T slice, moving, N=512)
bf16 matmuls accumulate fp32 into 2 PSUM banks per tile-half (4 tile
groups in flight). The W second-half DMA triggers and dequants are
woven into pass A's eviction stream, far off the critical path. All
dequant runs on DVE (GPSIMD tensor_tensor measured 2.5x-slowing
concurrent DVE ops; ACT's scale operand can't vary along free dims).
DVE/ACT split the PSUM eviction per chunk; each tile-half is DMA'd out
from SBUF staging via one gpsimd-issued trigger, and the last tile-half
is chunk-pipelined to shrink the drain tail.
"""

import os
import sys

import numpy as np

for _p in ("/opt/trn_rl_repo", "/root/.axon_site/_ro/trn_rl_repo"):
    if os.path.isdir(_p) and _p not in sys.path:
        sys.path.append(_p)

TOKENS, IN_F, OUT_F = 16384, 2048, 2048
NCORES = 8
TSH = TOKENS // NCORES  # tokens per core
P = 128
KB = IN_F // P  # contraction blocks
TB = TSH // P  # token tiles per core
OBL = OUT_F // P  # out_features blocks (scale granularity)
NCH = OUT_F // 512  # psum chunks of the output row-tile

_cached = None


def _build():
    from contextlib import ExitStack

    import concourse.tile as tile
    from concourse import bacc, mybir
    from concourse.bass import ds

    f32 = mybir.dt.float32
    bf16 = mybir.dt.bfloat16

    nc = bacc.Bacc("TRN2", target_bir_lowering=False, debug=False, num_devices=NCORES)
    xb_d = nc.dram_tensor("xb", [TB * P, IN_F], bf16, kind="ExternalInput").ap()
    wt_d = nc.dram_tensor("wt", [IN_F, OUT_F], bf16, kind="ExternalInput").ap()
    s_d = nc.dram_tensor("s", [P, KB, OBL], f32, kind="ExternalInput").ap()
    o_d = nc.dram_tensor("out", [TSH, OUT_F], f32, kind="ExternalOutput").ap()

    with tile.TileContext(nc) as tc:
        with ExitStack() as ctx:
            const = ctx.enter_context(tc.tile_pool(name="const", bufs=1))
            scales = const.tile([P, KB, OBL], f32)
            nc.scalar.dma_start(scales[:], s_d[:])

            wT_pool = ctx.enter_context(tc.tile_pool(name="wT", bufs=1))
            wTs = [wT_pool.tile([P, OUT_F], bf16, name=f"wT_{ib}") for ib in range(KB)]

            wnat_pool = ctx.enter_context(tc.tile_pool(name="wnat", bufs=6))
            x_pool = ctx.enter_context(tc.tile_pool(name="x", bufs=1))
            outsb_pool = ctx.enter_context(tc.tile_pool(name="outsb", bufs=3))
            ps_pool = ctx.enter_context(tc.tile_pool(name="ps", bufs=8, space="PSUM"))

            HW = OUT_F // 2  # output columns per pass

            def dequant(kb, src, lo, w):
                # wTs[kb][:, lo:lo+w] = src[:, 0:w] * scale; src 2D contiguous
                nb = w // P
                nc.vector.tensor_tensor(
                    out=wTs[kb][:, ds(lo, w)].rearrange("p (b c) -> p b c", c=P),
                    in0=src[:, ds(0, w)].rearrange("p (b c) -> p b c", c=P),
                    in1=scales[:, kb, ds(lo // P, nb), None].broadcast_to([P, nb, P]),
                    op=mybir.AluOpType.mult,
                )

            def emit_w_half(kb, half, trig, chunks=1):
                # one W row's half for one pass; trigger costs ~0.7us of
                # issuing-engine time, so pass-A triggers split sync/scalar
                wnat = wnat_pool.tile([P, HW], bf16, tag="wnat", name=f"wn_{kb}_{half}")
                cw = HW // chunks
                for j in range(chunks):
                    trig.dma_start(
                        wnat[:, ds(j * cw, cw)],
                        wt_d[ds(kb * P, P), ds(half * HW + j * cw, cw)],
                    )
                    dequant(kb, wnat[:, ds(j * cw, cw)], half * HW + j * cw, cw)

            xtiles = {}

            def emit_x_tile(t):
                xt = x_pool.tile([P, IN_F], bf16, name=f"x_{t}")
                nc.sync.dma_start(xt[:], xb_d[ds(t * P, P), :])
                xtiles[t] = xt

            psums = {}

            def open_group(t):
                psums[t] = [
                    ps_pool.tile([P, 512], f32, tag="ps", name=f"ps_{t}_{c}")
                    for c in range(2)
                ]

            def mm_one(t, kb, half, c):
                nc.tensor.matmul(
                    psums[t][c][:],
                    lhsT=xtiles[t][:, ds(kb * P, P)],
                    rhs=wTs[kb][:, ds(half * HW + c * 512, 512)],
                    start=(kb == 0),
                    stop=(kb == KB - 1),
                )

            def mm(t, kb, half):
                mm_one(t, kb, half, 0)
                mm_one(t, kb, half, 1)

            def close_tile(t, half, last=False):
                outsb = outsb_pool.tile([P, HW], f32, tag="osb", name=f"osb_{t}_{half}")
                for c in range(2):
                    if c == 0:
                        nc.vector.tensor_copy(outsb[:, ds(0, 512)], psums[t][0][:])
                    else:
                        nc.scalar.copy(outsb[:, ds(512, 512)], psums[t][1][:])
                    if last:
                        eng = nc.gpsimd if c == 0 else nc.scalar
                        eng.dma_start(
                            o_d[ds(t * P, P), ds(half * HW + c * 512, 512)],
                            outsb[:, ds(c * 512, 512)],
                        )
                if not last:
                    nc.gpsimd.dma_start(o_d[ds(t * P, P), ds(half * HW, HW)], outsb[:])
                del psums[t]

            # ---- prologue: x tiles 0-3 and pass-A W halves interleave on
            # the sync queue (even rows) and scalar queue (odd rows); row 0
            # is chunked so the first matmul starts as early as possible.
            emit_x_tile(0)
            emit_w_half(0, 0, nc.sync, chunks=2)
            emit_x_tile(1)
            for kb in range(1, KB):
                if kb % 2 == 0:
                    emit_w_half(kb, 0, nc.sync)
                else:
                    emit_w_half(kb, 0, nc.scalar)
                if kb == 1:
                    emit_x_tile(2)
                elif kb == 3:
                    emit_x_tile(3)
            for t in range(4, TB):
                emit_x_tile(t)

            # ---- pass A over out[:, 0:1024]: first four tiles interleaved
            # k-block-by-k-block (8 matmuls = ~1.7us of PE work per arriving
            # W half-row) so the PE never starves during the W load phase.
            for t in range(4):
                open_group(t)
            for kb in range(KB):
                for t in range(4):
                    mm(t, kb, 0)
            def weave_wb(t):
                # weave pass-B W DMAs + dequants into pass A's eviction
                # stream, two rows per tile close so all 16 are dequanted
                # well before pass B begins (~30us of slack)
                for kb in (2 * t, 2 * t + 1):
                    if kb < KB:
                        emit_w_half(kb, 1, nc.scalar)

            for t in range(4):
                close_tile(t, 0)
                weave_wb(t)

            for t in range(4, TB):
                open_group(t)
                for kb in range(KB):
                    mm(t, kb, 0)
                close_tile(t, 0)
                weave_wb(t)

            # ---- pass B over out[:, 1024:2048]: pure matmul stream ----
            for t in range(TB):
                open_group(t)
                last = t == TB - 1
                if not last:
                    for kb in range(KB):
                        mm(t, kb, 1)
                    close_tile(t, 1)
                else:
                    # chunk-outer on the final tile so the drain pipelines
                    outsb = outsb_pool.tile([P, HW], f32, tag="osb", name="osb_last")
                    for c in range(2):
                        for kb in range(KB):
                            mm_one(t, kb, 1, c)
                        if c == 0:
                            nc.vector.tensor_copy(
                                outsb[:, ds(0, 512)], psums[t][0][:]
                            )
                            nc.gpsimd.dma_start(
                                o_d[ds(t * P, P), ds(HW, 512)], outsb[:, ds(0, 512)]
                            )
                        else:
                            nc.scalar.copy(outsb[:, ds(512, 512)], psums[t][1][:])
                            nc.scalar.dma_start(
                                o_d[ds(t * P, P), ds(HW + 512, 512)],
                                outsb[:, ds(512, 512)],
                            )
                    del psums[t]

    nc.compile()
    return nc


def _get_compiled():
    global _cached
    if _cached is None:
        _cached = _build()
    return _cached


def _ensure_ntff_hook():
    """Register the axon NTFF profile hook (boot skips it when
    antenv.axon_hooks is absent from the image). Only needed for trace=True."""
    import sys as _sys
    import types as _types

    if "antenv.axon_hooks" not in _sys.modules:
        import antenv

        mod = _types.ModuleType("antenv.axon_hooks")
        mod._hook = None

        def set_axon_ntff_profile_hook(h):
            mod._hook = h

        def get_axon_ntff_profile_hook():
            return mod._hook

        mod.set_axon_ntff_profile_hook = set_axon_ntff_profile_hook
        mod.get_axon_ntff_profile_hook = get_axon_ntff_profile_hook
        _sys.modules["antenv.axon_hooks"] = mod
        antenv.axon_hooks = mod
    mod = _sys.modules["antenv.axon_hooks"]
    if mod._hook is None:
        from trn_agent_boot.trn_boot import _ntff_profile_via_ctypes

        hook = _ntff_profile_via_ctypes("/opt/axon/libaxon_pjrt.so")
        if hook is not None:
            mod.set_axon_ntff_profile_hook(hook)


def run(x, weight, weight_scale, trace=False, trace_cores=None):
    from concourse.bass_utils import run_bass_kernel_spmd

    nc = _get_compiled()

    import ml_dtypes

    bf16 = ml_dtypes.bfloat16
    x = np.asarray(x, dtype=np.float32)
    weight = np.asarray(weight, dtype=np.float32)
    wt = np.ascontiguousarray(weight.T.astype(bf16))
    weight_scale = np.asarray(weight_scale, dtype=np.float32)
    # [P, KB(bi), OBL(bo)]: s[p, bi, bo] = weight_scale[bo, bi]
    scales_b = np.ascontiguousarray(
        np.broadcast_to(weight_scale.T[None, :, :], (P, KB, OBL)).astype(np.float32)
    )

    def blocked_x(shard):
        # xb[t, p, kb, m] = shard[t*128+m, kb*128+p]  (layout prep only)
        xb = shard.reshape(TB, P, KB, P).transpose(0, 3, 2, 1)
        return np.ascontiguousarray(xb.astype(bf16).reshape(TB * P, IN_F))

    in_maps = [
        {
            "xb": blocked_x(x[c * TSH : (c + 1) * TSH]),
            "wt": wt,
            "s": scales_b,
        }
        for c in range(NCORES)
    ]
    kwargs = {}
    if trace:
        try:
            _ensure_ntff_hook()
        except Exception as e:  # tracing is best-effort; the run still works
            print(f"ntff hook registration failed ({e}); tracing may be skipped")
        kwargs = dict(trace=True, trace_cores=trace_cores or [0])
    res = run_bass_kernel_spmd(nc, in_maps, core_ids=list(range(NCORES)), **kwargs)
    out = np.concatenate([res.results[c]["out"] for c in range(NCORES)], axis=0)
    return out, res


def kernel(x, weight, weight_scale):
    # Rare transient device errors (NRT_EXEC_UNIT_UNRECOVERABLE) have been
    # observed under the profiling path; retry once to be safe.
    try:
        out, _ = run(x, weight, weight_scale)
    except Exception:
        import time

        time.sleep(2)
        out, _ = run(x, weight, weight_scale)
    return out
